# revision 1
# baseline (speedup 1.0000x reference)
"""GNN message passing (gather + weighted segment-sum) on 8 Trainium2 cores.

out[n, :] = sum_{e : dst[e] == n} weight[e] * queue[src[e], :]

Strategy
--------
Edges are sharded by destination window (128 destination nodes per window,
49 windows per core).  Each core:
  * gathers queue[src] rows straight from HBM with `dma_gather`
    (indices are int16, so the 50000-row queue is addressed as two
    parity-interleaved 25000-row strided views: even rows / odd rows),
  * builds a weighted one-hot matrix H[e, j] = weight[e] * (dstoff[e] == j)
    per 128-edge block with a single dual-op tensor_scalar on the DVE,
  * accumulates H.T @ G into a [128, 64] PSUM tile per window on the
    TensorEngine (fp32),
  * copies each finished window to SBUF and DMAs it to its slice of the
    output.

All data-dependent structure (edges per window, padded uniformly across
cores so one SPMD NEFF serves all 8 cores) is computed on the host from the
actual inputs at call time.
"""

import contextlib
import sys

sys.path.insert(0, "/opt/trn_rl_repo")

import ml_dtypes
import numpy as np

import concourse.bass as bass  # noqa: F401
import concourse.mybir as mybir
import concourse.tile as tile
from concourse import bacc
from concourse.bass_utils import run_bass_kernel_spmd

P = 128
N_CORES = 8

N_NODES = 50000
N_EDGES = 800000
D_FEAT = 64


def _plan(n_nodes, n_cores):
    """Windows-per-core and chunking. All cores run the identical program."""
    n_windows = -(-n_nodes // P)
    wpc = -(-n_windows // n_cores)
    # chunk width: largest divisor of wpc that keeps gather tiles a sane size
    cw = max(d for d in range(1, min(wpc, 8) + 1) if wpc % d == 0)
    nchunk = wpc // cw
    return wpc, cw, nchunk


def _host_prep(weight, src, dst, n_nodes, wpc, cw, nchunk, n_cores):
    """Bucket edges by (core, window, src parity); pad uniformly.

    Returns (epw, idx_hbm, aux_hbm):
      idx_hbm [n_cores, nchunk, 2, 128, cw*epw//16] int16  (dma_gather layout)
      aux_hbm [n_cores, nchunk, 128, 4*cw*nb] f32  (dstoff then weight, packed
              so block k of half h of window j sits at column (h*cw+j)*nb+k)
    """
    e = src.shape[0]
    src = np.asarray(src).astype(np.int64).reshape(-1)
    dst = np.asarray(dst).astype(np.int64).reshape(-1)
    wgt = np.asarray(weight, dtype=np.float32).reshape(-1)

    w = dst >> 7
    core = w // wpc
    lw = w - core * wpc
    half = src & 1
    hidx = (src >> 1).astype(np.int16)
    dstoff = (dst & 127).astype(np.float32)

    nbuckets = n_cores * wpc * 2
    key = (core * wpc + lw) * 2 + half
    # secondary sort key: src, for HBM locality within each gather
    order = np.lexsort((src, key))
    counts = np.bincount(key, minlength=nbuckets)
    epw = int(-(-max(int(counts.max()), 1) // P) * P)
    offs = np.zeros(nbuckets + 1, np.int64)
    np.cumsum(counts, out=offs[1:])
    skey = key[order]
    rank = np.arange(e, dtype=np.int64) - offs[skey]
    dest = skey * epw + rank

    # weight split: wgt == w_hi + w_lo with both terms bf16-exact
    w_hi = wgt.astype(ml_dtypes.bfloat16)
    w_lo = (wgt - w_hi.astype(np.float32)).astype(ml_dtypes.bfloat16)

    bf = ml_dtypes.bfloat16
    # pads are trailing -1 indices: the gather ucode trims them (no packets)
    idx_arr = np.full(nbuckets * epw, -1, np.int16)
    dst_arr = np.zeros(nbuckets * epw, bf)
    whi_arr = np.zeros(nbuckets * epw, bf)
    wlo_arr = np.zeros(nbuckets * epw, bf)
    idx_arr[dest] = hidx[order]
    dst_arr[dest] = dstoff[order].astype(bf)  # 0..127, exact in bf16
    whi_arr[dest] = w_hi[order]
    wlo_arr[dest] = w_lo[order]

    nb = epw // P
    big = cw * epw  # indices per chunk-half
    shp = (n_cores, nchunk, cw, 2, epw)
    idx_arr = idx_arr.reshape(shp)
    dst_arr = dst_arr.reshape(shp)
    whi_arr = whi_arr.reshape(shp)
    wlo_arr = wlo_arr.reshape(shp)

    # idx: window-major edge list per (core, chunk, half), wrapped mod 16 and
    # replicated to 128 partitions (8 Q7 cores each read a 16-partition copy).
    a = idx_arr.transpose(0, 1, 3, 2, 4).reshape(n_cores, nchunk, 2, big // 16, 16)
    a = a.transpose(0, 1, 2, 4, 3)  # [.., 16, big//16]
    idx_hbm = np.broadcast_to(
        a[:, :, :, None, :, :], (n_cores, nchunk, 2, 8, 16, big // 16)
    ).reshape(n_cores, nchunk, 2, P, big // 16)
    idx_hbm = np.ascontiguousarray(idx_hbm)

    def pack(x):
        # window-major block columns: col = (j*2 + h)*nb + k
        y = x.reshape(n_cores, nchunk, cw, 2, nb, P)
        y = y.transpose(0, 1, 5, 2, 3, 4)  # [core, chunk, P, j, h, k]
        return y.reshape(n_cores, nchunk, P, 2 * cw * nb)

    aux_hbm = np.concatenate(
        [pack(dst_arr), pack(whi_arr), pack(wlo_arr)], axis=3
    )
    aux_hbm = np.ascontiguousarray(aux_hbm)
    # per-gather valid-edge counts, ordered (chunk, window, half)
    cnt_hbm = np.ascontiguousarray(
        counts.reshape(n_cores, nchunk, cw, 2)
        .reshape(n_cores, 1, nchunk * cw * 2)
        .astype(np.int32)
    )
    return epw, idx_hbm, aux_hbm, cnt_hbm


ALL_PARTS = frozenset({"gather", "dve", "mm", "out"})

TERMS = 3  # hi*hi + hi*lo + lo*hi (lo*lo ~ 4e-6 relative, dropped)


def _build(n_nodes, d, epw, wpc, cw, nchunk, iters=1, parts=ALL_PARTS):
    f32 = mybir.dt.float32
    bf16 = mybir.dt.bfloat16
    nb = epw // P
    big = cw * epw
    bpc = cw * nb  # blocks per half per chunk
    ne = n_nodes // 2
    assert n_nodes % 2 == 0

    nc = bacc.Bacc(
        "TRN2", target_bir_lowering=False, debug=False, num_swdge_queues=4
    )

    # qhl[p] = 256 bf16: [hi(node 2p) | lo(node 2p) | hi(node 2p+1) | lo(node 2p+1)]
    qhl_t = nc.dram_tensor("qhl", [ne, 4 * d], bf16, kind="ExternalInput")
    idx_t = nc.dram_tensor(
        "idx", [nchunk, 2, P, big // 16], mybir.dt.int16, kind="ExternalInput"
    )
    aux_t = nc.dram_tensor("aux", [nchunk, P, 6 * bpc], bf16, kind="ExternalInput")
    iota_t = nc.dram_tensor("iota", [P, P], bf16, kind="ExternalInput")
    cnt_t = nc.dram_tensor(
        "cnt", [1, nchunk * cw * 2], mybir.dt.int32, kind="ExternalInput"
    )
    out_t = nc.dram_tensor("out", [wpc * P, d], f32, kind="ExternalOutput")

    q2 = qhl_t.ap()  # [ne, 4d]
    qviews = [q2[:, 0 : 2 * d], q2[:, 2 * d : 4 * d]]

    with tile.TileContext(nc) as tc:
        gbufs = 6
        with (
            tc.tile_pool(name="const", bufs=1) as cpool,
            tc.tile_pool(name="io", bufs=2) as iopool,
            tc.tile_pool(name="gat", bufs=gbufs) as gpool,
            tc.tile_pool(name="hot", bufs=3) as hpool,
            tc.tile_pool(name="ost", bufs=4) as opool,
            tc.tile_pool(name="ps", bufs=4, space="PSUM") as ppool,
        ):
            iota_f = cpool.tile([P, P], bf16)
            nc.sync.dma_start(out=iota_f[:], in_=iota_t.ap()[:, :])
            cnt = cpool.tile([1, nchunk * cw * 2], mybir.dt.int32)
            nc.sync.dma_start(out=cnt[:], in_=cnt_t.ap()[:, :])
            # pre-zero the gather slots: trimmed (padded) tail positions are
            # never written by the gather, and must not contain NaN patterns
            for h in (0, 1):
                for _ in range(gbufs):
                    gz = gpool.tile([P, nb, 2 * d], bf16, tag=f"g{h}")
                    nc.vector.memset(gz[:], 0)

            loop = tc.For_i(0, iters, 1) if iters > 1 else contextlib.nullcontext()
            with loop:
                for c in range(nchunk):
                    idxs = []
                    for h in (0, 1):
                        it = iopool.tile(
                            [P, big // 16], mybir.dt.int16, tag=f"idx{h}"
                        )
                        nc.sync.dma_start(out=it[:], in_=idx_t.ap()[c, h])
                        idxs.append(it)
                    aux = iopool.tile([P, 6 * bpc], bf16, tag="aux")
                    nc.sync.dma_start(out=aux[:], in_=aux_t.ap()[c])

                    for j in range(cw):
                        gt = []
                        for h in (0, 1):
                            g = gpool.tile([P, nb, 2 * d], bf16, tag=f"g{h}")
                            if "gather" in parts:
                                sl = epw // 16
                                gidx = (c * cw + j) * 2 + h
                                r = nc.alloc_register(mybir.EngineType.Pool)
                                nc.gpsimd.reg_load(
                                    r, cnt[0:1, gidx : gidx + 1]
                                )
                                nc.gpsimd.dma_gather(
                                    out_ap=g[:],
                                    in_ap=qviews[h],
                                    idxs_ap=idxs[h][:, j * sl : (j + 1) * sl],
                                    num_idxs=epw,
                                    num_idxs_reg=r,
                                    elem_size=2 * d,
                                    elem_step=4 * d,
                                    single_packet=False,
                                    queue_num=(2 * j + h) % 4,
                                )
                            elif "seqload" in parts:
                                flat = qhl_t.ap()[0 : P * 64, :].rearrange(
                                    "(p c) d -> p (c d)", p=P
                                )
                                nc.sync.dma_start(
                                    out=g[:].rearrange("p a b -> p (a b)"),
                                    in_=flat[:, 0 : nb * 2 * d],
                                )
                            gt.append(g)

                        ps = ppool.tile([P, d], f32)
                        nbw = 2 * nb  # blocks in this window (both halves)
                        wcol = j * nbw  # first block column of this window

                        def bcast(ap2d, n_mid, mid_is_data):
                            # [P, X] -> [P, n_mid, P] AP; data dim keeps its
                            # step, the other dim gets step 0
                            pairs = list(ap2d.ap)
                            assert len(pairs) == 2
                            if mid_is_data:
                                newp = [pairs[0], [pairs[1][0], n_mid], [0, P]]
                            else:
                                newp = [pairs[0], [0, n_mid], pairs[1]]
                            return bass.AP(ap2d.tensor, ap2d.offset, newp)

                        if "dve" in parts:
                            h01 = hpool.tile([P, nbw, P], bf16, tag="h01")
                            nc.vector.tensor_tensor(
                                out=h01[:],
                                in0=bcast(iota_f[:], nbw, False),
                                in1=bcast(aux[:, wcol : wcol + nbw], nbw, True),
                                op=mybir.AluOpType.is_equal,
                            )
                            hhi = hpool.tile([P, nbw, P], bf16, tag="hhi")
                            nc.vector.tensor_tensor(
                                out=hhi[:],
                                in0=h01[:],
                                in1=bcast(
                                    aux[:, 2 * bpc + wcol : 2 * bpc + wcol + nbw],
                                    nbw,
                                    True,
                                ),
                                op=mybir.AluOpType.mult,
                            )
                            hlo = hpool.tile([P, nbw, P], bf16, tag="hlo")
                            nc.vector.tensor_tensor(
                                out=hlo[:],
                                in0=h01[:],
                                in1=bcast(
                                    aux[:, 4 * bpc + wcol : 4 * bpc + wcol + nbw],
                                    nbw,
                                    True,
                                ),
                                op=mybir.AluOpType.mult,
                            )
                        if "mm" in parts:
                            first = True
                            for h in (0, 1):
                                for k in range(nb):
                                    bi = h * nb + k  # block within window
                                    if "dve" in parts:
                                        hi_ap = hhi[:, bi, :]
                                        lo_ap = hlo[:, bi, :]
                                    else:
                                        hi_ap = lo_ap = iota_f[:]
                                    g_hi = gt[h][:, k, 0:d]
                                    g_lo = gt[h][:, k, d : 2 * d]
                                    terms = [
                                        (hi_ap, g_hi),
                                        (hi_ap, g_lo),
                                        (lo_ap, g_hi),
                                    ]
                                    if TERMS == 4:
                                        terms.append((lo_ap, g_lo))
                                    last_blk = h == 1 and k == nb - 1
                                    for t, (lhs, rhs) in enumerate(terms):
                                        nc.tensor.matmul(
                                            ps[:],
                                            lhsT=lhs,
                                            rhs=rhs,
                                            start=first,
                                            stop=last_blk
                                            and t == len(terms) - 1,
                                        )
                                        first = False
                        wg = c * cw + j
                        if "out" in parts and "mm" in parts:
                            ot = opool.tile([P, d], f32, tag="ot")
                            nc.scalar.copy(ot[:], ps[:])
                            nc.sync.dma_start(
                                out=out_t.ap()[wg * P : (wg + 1) * P, :], in_=ot[:]
                            )
                        elif "dve" in parts and "mm" not in parts:
                            # variant build: keep the one-hots alive (anti-DCE)
                            nc.vector.tensor_tensor(
                                out=hhi[:, 0, :],
                                in0=hhi[:, 0, :],
                                in1=hlo[:, 0, :],
                                op=mybir.AluOpType.max,
                            )
                            nc.sync.dma_start(
                                out=out_t.ap()[wg * P : (wg + 1) * P, :],
                                in_=hhi[:, 0, :].bitcast(f32),
                            )
                        elif "dve" not in parts and "mm" not in parts:
                            # variant build: keep the loads alive (anti-DCE)
                            nc.sync.dma_start(
                                out=out_t.ap()[wg * P : (wg + 1) * P, :],
                                in_=gt[0][:, 0, :].bitcast(f32),
                            )
                            nc.sync.dma_start(
                                out=out_t.ap()[wg * P : (wg + 1) * P, :],
                                in_=gt[1][:, 0, :].bitcast(f32),
                            )
    nc.compile()
    return nc


def _make_inputs(queue, idx_hbm, aux_hbm, cnt_hbm, n_cores):
    bf = ml_dtypes.bfloat16
    q = np.asarray(queue, dtype=np.float32)
    hi = q.astype(bf)
    lo = (q - hi.astype(np.float32)).astype(bf)
    ne, d = q.shape[0] // 2, q.shape[1]
    qhl = np.empty((ne, 4 * d), bf)
    qhl[:, 0:d] = hi[0::2]
    qhl[:, d : 2 * d] = lo[0::2]
    qhl[:, 2 * d : 3 * d] = hi[1::2]
    qhl[:, 3 * d : 4 * d] = lo[1::2]
    iota_np = np.ascontiguousarray(
        np.broadcast_to(np.arange(P, dtype=np.float32), (P, P)).astype(bf)
    )
    return [
        {
            "qhl": qhl,
            "idx": idx_hbm[c],
            "aux": aux_hbm[c],
            "iota": iota_np,
            "cnt": cnt_hbm[c],
        }
        for c in range(n_cores)
    ]


def _run(queue, weight, src, dst, n_nodes, d, n_cores, trace=False, iters=1):
    queue = np.ascontiguousarray(np.asarray(queue, dtype=np.float32))
    wpc, cw, nchunk = _plan(n_nodes, n_cores)
    epw, idx_hbm, aux_hbm, cnt_hbm = _host_prep(
        weight, src, dst, n_nodes, wpc, cw, nchunk, n_cores
    )
    nc = _build(n_nodes, d, epw, wpc, cw, nchunk, iters=iters)
    in_maps = _make_inputs(queue, idx_hbm, aux_hbm, cnt_hbm, n_cores)
    res = run_bass_kernel_spmd(nc, in_maps, core_ids=list(range(n_cores)), trace=trace)
    full = np.concatenate([res.results[c]["out"] for c in range(n_cores)], axis=0)
    return full[:n_nodes], res


def kernel(queue, weight, src, dst):
    out, _ = _run(queue, weight, src, dst, N_NODES, D_FEAT, N_CORES)
    return out



# revision 2
# speedup vs baseline: 1.5921x; 1.5921x over previous
"""GNN message passing (gather + weighted segment-sum) on 8 Trainium2 cores.

out[n, :] = sum_{e : dst[e] == n} weight[e] * queue[src[e], :]

Design
------
Edges are sharded by destination window (128 dst nodes per window, 49
windows per core), bucketed per (window, src-parity) and sorted by src.
The queue is stored in HBM as fp16 node-PAIRS: qp[t] = [fp16(queue[2t]) |
fp16(queue[2t+1])] (256B rows — the dma_gather element-size floor), so an
edge's row index is src>>1 (fits int16) and the parity picks the useful
64-value half.  Each core per iteration:
  * window-level dma_gathers of qp rows per (chunk, window, parity) —
    pads point at row 0 and carry weight 0,
  * one DVE pass per (chunk, parity) folds the edge weight into the
    gathered rows: gw = g[:, parity-half] * w  (fp16),
  * one DVE is_equal pass per window builds the pure one-hot H01 from
    iota vs per-edge dst offsets,
  * ONE matmul per 128-edge block: ps[128 dst, 64] += H01_blk^T @ gw_blk
    (vs 3 matmul terms in the hi/lo-bf16 formulation),
  * ACT copies ps -> SBUF, DMA to the output slice.

PSUM accumulator tiles are padded to full 2KB banks: a start=True matmul
clears the whole bank's has_written bits, so accumulators must not share
banks with other matmul groups.

All data-dependent structure is computed on the host from the actual
inputs at call time; all 8 cores run one SPMD program.
"""

import contextlib
import sys

sys.path.insert(0, "/opt/trn_rl_repo")

import ml_dtypes  # noqa: F401
import numpy as np

import concourse.bass as bass  # noqa: F401
import concourse.mybir as mybir
import concourse.tile as tile
from concourse import bacc
from concourse.bass_utils import run_bass_kernel_spmd

P = 128
N_CORES = 8

N_NODES = 50000
N_EDGES = 800000
D_FEAT = 64

NPAIR = N_NODES // 2  # fp16 pair-rows in qp
RANKS = -(-NPAIR // P)  # stripe columns in the SBUF-resident queue
NPAD = RANKS * P  # qp rows incl. padding (rank grid)


def _plan(n_nodes, n_cores):
    n_windows = -(-n_nodes // P)
    wpc = -(-n_windows // n_cores)
    cw = max(d for d in range(1, min(wpc, 8) + 1) if wpc % d == 0)
    nchunk = wpc // cw
    return wpc, cw, nchunk


def _host_prep(weight, src, dst, n_nodes, wpc, cw, nchunk, n_cores):
    """Bucket edges by (core, window, src parity); pad uniformly.

    Returns (epw, idx_hbm, aux_hbm):
      idx_hbm [n_cores, nchunk, 2, 128, cw*epw//16] int16 (dma_gather layout,
              pads -> row 0)
      aux_hbm [n_cores, nchunk, 128, 4*cw*nb] fp16:
              cols [0, 2*cw*nb)         dstoff, window-major (j*2+h)*nb+k
              cols [2*cw*nb, 4*cw*nb)   weight, half-major   h*cw*nb+j*nb+k
    """
    e = src.shape[0]
    src = np.asarray(src).astype(np.int64).reshape(-1)
    dst = np.asarray(dst).astype(np.int64).reshape(-1)
    wgt = np.asarray(weight, dtype=np.float32).reshape(-1)

    w = dst >> 7
    core = w // wpc
    lw = w - core * wpc
    half = src & 1
    hidx = (src >> 1).astype(np.int16)
    dstoff = (dst & 127).astype(np.float32)

    nbuckets = n_cores * wpc * 2
    key = (core * wpc + lw) * 2 + half
    order = np.lexsort((src, key))
    counts = np.bincount(key, minlength=nbuckets)
    epw = int(-(-max(int(counts.max()), 1) // P) * P)
    offs = np.zeros(nbuckets + 1, np.int64)
    np.cumsum(counts, out=offs[1:])
    skey = key[order]
    rank = np.arange(e, dtype=np.int64) - offs[skey]
    dest = skey * epw + rank

    f16 = np.float16
    idx_arr = np.zeros(nbuckets * epw, np.int16)  # pads gather row 0
    dst_arr = np.zeros(nbuckets * epw, f16)
    wgt_arr = np.zeros(nbuckets * epw, f16)  # pads carry weight 0
    idx_arr[dest] = hidx[order]
    dst_arr[dest] = dstoff[order].astype(f16)  # 0..127, exact
    wgt_arr[dest] = wgt[order].astype(f16)

    nb = epw // P
    big = cw * epw
    shp = (n_cores, nchunk, cw, 2, epw)
    idx_arr = idx_arr.reshape(shp)
    dst_arr = dst_arr.reshape(shp)
    wgt_arr = wgt_arr.reshape(shp)

    # idx: half-major edge list per (core, chunk), window-major within a half,
    # wrapped mod 16 and replicated to 128 partitions.
    a = idx_arr.transpose(0, 1, 3, 2, 4).reshape(n_cores, nchunk, 2, big // 16, 16)
    a = a.transpose(0, 1, 2, 4, 3)
    idx_hbm = np.broadcast_to(
        a[:, :, :, None, :, :], (n_cores, nchunk, 2, 8, 16, big // 16)
    ).reshape(n_cores, nchunk, 2, P, big // 16)
    idx_hbm = np.ascontiguousarray(idx_hbm)

    def pack_wmaj(x):
        # window-major block columns: col = (j*2 + h)*nb + k
        y = x.reshape(n_cores, nchunk, cw, 2, nb, P)
        y = y.transpose(0, 1, 5, 2, 3, 4)  # [core, chunk, P, j, h, k]
        return y.reshape(n_cores, nchunk, P, 2 * cw * nb)

    def pack_hmaj(x):
        # half-major block columns: col = h*cw*nb + j*nb + k
        y = x.reshape(n_cores, nchunk, cw, 2, nb, P)
        y = y.transpose(0, 1, 5, 3, 2, 4)  # [core, chunk, P, h, j, k]
        return y.reshape(n_cores, nchunk, P, 2 * cw * nb)

    aux_hbm = np.concatenate([pack_wmaj(dst_arr), pack_hmaj(wgt_arr)], axis=3)
    aux_hbm = np.ascontiguousarray(aux_hbm)
    wgt32 = np.zeros(nbuckets * epw, np.float32)
    wgt32[dest] = wgt[order]
    wgt32 = wgt32.reshape(shp)
    y = wgt32.reshape(n_cores, nchunk, cw, 2, nb, P)
    y = y.transpose(0, 1, 5, 3, 2, 4)  # half-major, matches aux weight cols
    wgt_hbm = np.ascontiguousarray(y.reshape(n_cores, nchunk, P, 2 * cw * nb))
    cnt_hbm = np.ascontiguousarray(
        counts.reshape(n_cores, nchunk, cw, 2)
        .transpose(0, 1, 3, 2)  # [core, chunk, half, window]
        .reshape(n_cores, 1, nchunk * 2 * cw)
        .astype(np.int32)
    )
    return epw, idx_hbm, aux_hbm, wgt_hbm, cnt_hbm


ALL_PARTS = frozenset({"gather", "dve", "mm", "out"})

KSB = (0, 0)  # SBUF-path window count; 0 = all edges via HBM gather
# (the SBUF-resident transpose-gather path computes correctly in isolation
# but its RX completion is under-synchronized in integration; disabled)


def _build(
    n_nodes, d, epw, wpc, cw, nchunk, iters=1, parts=ALL_PARTS, nq=4, ksb=KSB,
    trim=False,
):
    f32 = mybir.dt.float32
    f16 = mybir.dt.float16
    nb = epw // P
    big = cw * epw
    cwnb = cw * nb
    assert n_nodes % 2 == 0

    nc = bacc.Bacc(
        "TRN2", target_bir_lowering=False, debug=False, num_swdge_queues=nq
    )

    qp_t = nc.dram_tensor("qp", [NPAD, 2 * d], f16, kind="ExternalInput")
    qps_t = nc.dram_tensor("qps", [P, RANKS * 2 * d], f16, kind="ExternalInput")
    idx_t = nc.dram_tensor(
        "idx", [nchunk, 2, P, big // 16], mybir.dt.int16, kind="ExternalInput"
    )
    aux_t = nc.dram_tensor("aux", [nchunk, P, 4 * cwnb], f16, kind="ExternalInput")
    wgt_t = nc.dram_tensor("wgt", [nchunk, P, 2 * cwnb], f32, kind="ExternalInput")
    cnt_t = nc.dram_tensor(
        "cnt", [1, nchunk * 2 * cw], mybir.dt.int32, kind="ExternalInput"
    )
    iota_t = nc.dram_tensor("iota", [P, P], f16, kind="ExternalInput")
    ident_t = nc.dram_tensor("ident", [P, P], f16, kind="ExternalInput")
    out_t = nc.dram_tensor("out", [wpc * P, d], f32, kind="ExternalOutput")

    qview = qp_t.ap()[:, 0 : 2 * d]

    def bcast(ap2d, n_mid, mid_is_data):
        pairs = list(ap2d.ap)
        assert len(pairs) == 2
        if mid_is_data:
            newp = [pairs[0], [pairs[1][0], n_mid], [0, P]]
        else:
            newp = [pairs[0], [0, n_mid], pairs[1]]
        return bass.AP(ap2d.tensor, ap2d.offset, newp)

    def bcast64(ap2d, n_mid):
        pairs = list(ap2d.ap)
        assert len(pairs) == 2
        newp = [pairs[0], [pairs[1][0], n_mid], [0, 64]]
        return bass.AP(ap2d.tensor, ap2d.offset, newp)

    with tile.TileContext(nc) as tc:
        with (
            tc.tile_pool(name="const", bufs=1) as cpool,
            tc.tile_pool(name="io", bufs=2) as iopool,
            tc.tile_pool(name="gat", bufs=2) as gpool,
            tc.tile_pool(name="gts", bufs=2) as tpool,
            tc.tile_pool(name="gw", bufs=2) as wpool,
            tc.tile_pool(name="gwsb", bufs=10) as spool,
            tc.tile_pool(name="hot", bufs=3) as hpool,
            tc.tile_pool(name="ost", bufs=4) as opool,
            tc.tile_pool(name="ps", bufs=4, space="PSUM") as ppool,
            tc.tile_pool(name="pst", bufs=4, space="PSUM") as ppool2,
        ):
            iota_f = cpool.tile([P, P], f16)
            nc.sync.dma_start(out=iota_f[:], in_=iota_t.ap()[:, :])
            ident = cpool.tile([P, P], f16)
            nc.sync.dma_start(out=ident[:], in_=ident_t.ap()[:, :])
            qsb = cpool.tile([P, RANKS * 2 * d], f16)
            cnt = cpool.tile([1, nchunk * 2 * cw], mybir.dt.int32)
            nc.sync.dma_start(out=cnt[:], in_=cnt_t.ap()[:, :])
            # pre-zero HBM gather slots: reg-trimmed tails are never written
            # and must not hold NaN bit patterns on the first iteration
            for h in (0, 1):
                if cw > ksb[h]:
                    for _ in range(2):
                        gz = gpool.tile(
                            [P, (cw - ksb[h]) * nb, 2 * d], f16, tag=f"g{h}"
                        )
                        nc.vector.memset(gz[:], 0)

            loop = tc.For_i(0, iters, 1) if iters > 1 else contextlib.nullcontext()
            with loop:
                # stage the queue into SBUF (stripe-major node-pair tokens)
                if "gather" in parts:
                    nc.sync.dma_start(out=qsb[:], in_=qps_t.ap()[:, :])
                for c in range(nchunk):
                    idxs = []
                    for h in (0, 1):
                        it = iopool.tile(
                            [P, big // 16], mybir.dt.int16, tag=f"idx{h}"
                        )
                        nc.sync.dma_start(out=it[:], in_=idx_t.ap()[c, h])
                        idxs.append(it)
                    aux = iopool.tile([P, 4 * cwnb], f16, tag="aux")
                    nc.sync.dma_start(out=aux[:], in_=aux_t.ap()[c])
                    wf = None
                    if max(ksb) > 0:
                        wf = iopool.tile([P, 2 * cwnb], f32, tag="wgt")
                        nc.sync.dma_start(out=wf[:], in_=wgt_t.ap()[c])

                    sl = epw // 16
                    ghbm = []
                    gws = []
                    gsb = {}
                    # HBM-path gathers first: they are the long pole
                    for h in (0, 1):
                        k = ksb[h]
                        g = None
                        if cw > k:
                            g = gpool.tile(
                                [P, (cw - k) * nb, 2 * d], f16, tag=f"g{h}"
                            )
                        if "gather" in parts and cw > k:
                            nqh = 2 if max(ksb) > 0 else nq
                            for j in range(k, cw):
                                jj = j - k
                                if trim:
                                    r = nc.alloc_register(mybir.EngineType.Pool)
                                    gi = (c * 2 + h) * cw + j
                                    nc.gpsimd.reg_load(r, cnt[0:1, gi : gi + 1])
                                else:
                                    r = epw
                                nc.gpsimd.dma_gather(
                                    out_ap=g[:, jj * nb : (jj + 1) * nb, :],
                                    in_ap=qview,
                                    idxs_ap=idxs[h][:, j * sl : (j + 1) * sl],
                                    num_idxs=epw,
                                    num_idxs_reg=r,
                                    elem_size=2 * d,
                                    elem_step=2 * d,
                                    single_packet=False,
                                    queue_num=(j + h) % nqh,
                                )
                        ghbm.append(g)
                    # SBUF-path gathers: windows [0, k), window-level
                    for j in range(max(ksb)):
                        for h in (0, 1):
                            if j >= ksb[h]:
                                continue
                            gt = tpool.tile([P, 1, epw], f16, tag=f"gt{h}{j}")
                            if "gather" in parts:
                                nc.gpsimd.dma_gather(
                                    out_ap=gt[:],
                                    in_ap=qsb[:],
                                    idxs_ap=idxs[h][:, j * sl : (j + 1) * sl],
                                    num_idxs=epw,
                                    num_idxs_reg=epw,
                                    elem_size=2 * d,
                                    transpose=True,
                                    single_packet=False,
                                    queue_num=2 + ((j + h) % 2),
                                    sbuf_tokens_per_rank=P,
                                    sbuf_free_dim_per_rank=4 * d,
                                    sbuf_free_dim_pad_per_rank=0,
                                    sbuf_byte_offset=0,
                                )
                            gsb[(h, j)] = gt
                    for h in (0, 1):
                        k = ksb[h]
                        gw = None
                        if cw > k:
                            gw = wpool.tile([P, (cw - k) * nb, d], f16, tag=f"gw{h}")
                        if "dve" in parts and cw > k:
                            # gw = g[:, :, h*64:(h+1)*64] * w (per edge)
                            nc.vector.tensor_tensor(
                                out=gw[:],
                                in0=ghbm[h][:, :, h * d : (h + 1) * d],
                                in1=bcast64(
                                    aux[
                                        :,
                                        (2 + h) * cwnb + k * nb : (3 + h) * cwnb,
                                    ],
                                    (cw - k) * nb,
                                ),
                                op=mybir.AluOpType.mult,
                            )
                        gws.append(gw)

                    for j in range(cw):
                        nbw = 2 * nb
                        wcol = j * nbw
                        h01 = hpool.tile([P, nbw, P], f16, tag="h01")
                        if "dve" in parts:
                            nc.vector.tensor_tensor(
                                out=h01[:],
                                in0=bcast(iota_f[:], nbw, False),
                                in1=bcast(aux[:, wcol : wcol + nbw], nbw, True),
                                op=mybir.AluOpType.is_equal,
                            )
                        # SBUF-path rhs blocks: transpose + weight on ACT
                        rhs_sb = {}
                        for h in (0, 1):
                            if j >= ksb[h] or "gather" not in parts:
                                continue
                            gt = gsb[(h, j)]
                            for k2 in range(nb):
                                # transpose as a plain matmul: gt_blk.T @ I
                                # (full-bank PSUM tile: start=True clears the
                                # whole bank's has_written bits, so no other
                                # accumulator may share this bank)
                                pt = ppool2.tile([P, 512], f32, tag="pt")
                                nc.tensor.matmul(
                                    pt[:, 0:P],
                                    lhsT=gt[:, 0, k2 * P : (k2 + 1) * P],
                                    rhs=ident[:],
                                    start=True,
                                    stop=True,
                                )
                                gwb = spool.tile([P, 64], f16, tag=f"gwb{h}")
                                wc = h * cwnb + j * nb + k2
                                nc.scalar.mul(
                                    gwb[:],
                                    pt[:, h * 64 : (h + 1) * 64],
                                    wf[:, wc : wc + 1],
                                )
                                rhs_sb[(h, k2)] = gwb
                        ps = ppool.tile([P, 512], f32)
                        if "mm" in parts:
                            first = True
                            for h in (0, 1):
                                k = ksb[h]
                                for k2 in range(nb):
                                    bi = h * nb + k2
                                    lhs = (
                                        h01[:, bi, :]
                                        if "dve" in parts
                                        else iota_f[:]
                                    )
                                    if j < k and "gather" in parts:
                                        rhs = rhs_sb[(h, k2)][:]
                                    else:
                                        jj = max(j - k, 0)
                                        rhs = gws[h][:, jj * nb + k2, :]
                                    nc.tensor.matmul(
                                        ps[:, 0:d],
                                        lhsT=lhs,
                                        rhs=rhs,
                                        start=first,
                                        stop=(h == 1 and k2 == nb - 1),
                                    )
                                    first = False
                        wg = c * cw + j
                        if "out" in parts and "mm" in parts:
                            ot = opool.tile([P, d], f32, tag="ot")
                            nc.scalar.copy(ot[:], ps[:, 0:d])
                            nc.sync.dma_start(
                                out=out_t.ap()[wg * P : (wg + 1) * P, :], in_=ot[:]
                            )
                        elif "mm" not in parts:
                            if "dve" in parts:
                                srcs = [h01[:, 0, :]]
                            else:
                                srcs = [
                                    g[:, 0, 0:P] for g in ghbm if g is not None
                                ]
                            for src_ap in srcs:
                                nc.sync.dma_start(
                                    out=out_t.ap()[wg * P : (wg + 1) * P, 0:32],
                                    in_=src_ap.bitcast(f32)[:, 0:32],
                                )
    nc.compile()
    return nc


def _make_inputs(queue, idx_hbm, aux_hbm, wgt_hbm, cnt_hbm, n_cores):
    f16 = np.float16
    q = np.asarray(queue, dtype=np.float32)
    d = q.shape[1]
    qp = np.zeros((NPAD, 2 * d), f16)
    qp[:NPAIR, 0:d] = q[0::2].astype(f16)
    qp[:NPAIR, d : 2 * d] = q[1::2].astype(f16)
    # stripe-major staging image: token t -> partition t & 127,
    # free cols (t>>7)*2d .. +2d
    qps = np.ascontiguousarray(
        qp.reshape(RANKS, P, 2 * d).transpose(1, 0, 2).reshape(P, RANKS * 2 * d)
    )
    iota_np = np.ascontiguousarray(
        np.broadcast_to(np.arange(P, dtype=np.float32), (P, P)).astype(f16)
    )
    ident_np = np.ascontiguousarray(np.eye(P, dtype=f16))
    return [
        {
            "qp": qp,
            "qps": qps,
            "idx": idx_hbm[c],
            "aux": aux_hbm[c],
            "wgt": wgt_hbm[c],
            "cnt": cnt_hbm[c],
            "iota": iota_np,
            "ident": ident_np,
        }
        for c in range(n_cores)
    ]


def _run(queue, weight, src, dst, n_nodes, d, n_cores, iters=1):
    queue = np.ascontiguousarray(np.asarray(queue, dtype=np.float32))
    wpc, cw, nchunk = _plan(n_nodes, n_cores)
    epw, idx_hbm, aux_hbm, wgt_hbm, cnt_hbm = _host_prep(
        weight, src, dst, n_nodes, wpc, cw, nchunk, n_cores
    )
    nc = _build(n_nodes, d, epw, wpc, cw, nchunk, iters=iters)
    in_maps = _make_inputs(queue, idx_hbm, aux_hbm, wgt_hbm, cnt_hbm, n_cores)
    res = run_bass_kernel_spmd(nc, in_maps, core_ids=list(range(n_cores)))
    full = np.concatenate([res.results[c]["out"] for c in range(n_cores)], axis=0)
    return full[:n_nodes], res


def kernel(queue, weight, src, dst):
    out, _ = _run(queue, weight, src, dst, N_NODES, D_FEAT, N_CORES)
    return out
